# revision 32
# baseline (speedup 1.0000x reference)
"""ChebGraphConv (K=3) Trainium2 kernel.

y = x@(W0-W2) - (A@x)@W1 + 2*A@((A@x)@W2) + bias

computed per (b,t) slice as:
  P0 = X@W02 ; P1 = X@W1 ; P2' = X@(2*W2)   (projections from X^T hi/lo bf16,
                                             one 256-wide matmul per node block)
  Q' = A@P2' ; M = P1 - Q' ; S = A@M ; y = P0 - S (+bias)

The two spmms run as fp8e4m3 DoubleRow matmuls (2.9x the bf16 rate): A^T is
host-packed as 2048*A in fp8 with the DoubleRow [Ki,2,*] plane = node
sub-block, and each spmm PSUM result is scaled by 1/2048 on the Scalar engine
before the subtract. All fp8/bf16 rounding lands on the small A@(...) terms
(|A@v| ~ 0.01*|v|); the dominant P0 term uses an X-hi/lo + W02-hi/lo bf16
split, keeping output error ~1e-4 relative.

Data parallel over B: core b handles x[b] (T=12 slices), 2 groups of 6 slices
so the spmm moving operand is [128, 2, 384].
"""

import numpy as np
import ml_dtypes

import concourse.bacc as bacc
import concourse.mybir as mybir
import concourse.tile as tile
from concourse import bass_utils

BF16 = ml_dtypes.bfloat16
FP8 = ml_dtypes.float8_e4m3

B, T, N, C = 8, 12, 2048, 64
NB = N // 128          # 16 node blocks
NMT = NB // 2          # 8 DoubleRow contraction tiles (256 nodes each)
GROUPS = (6, 6)
ASCALE = 2048.0        # fp8 A is stored as A*ASCALE (A entries ~U(0,1/2048))

_NC_CACHE = {}


def _build_nc(repeat=None, with_bias=False):
    """repeat=None: single-shot kernel (graded path). repeat=R: wraps the
    whole body in a hardware For loop running it R times (benchmarking)."""
    key = ("nc", repeat, with_bias)
    if key in _NC_CACHE:
        return _NC_CACHE[key]
    f32 = mybir.dt.float32
    bf16 = mybir.dt.bfloat16
    fp8 = mybir.dt.float8e4

    nc = bacc.Bacc("TRN2", target_bir_lowering=False, debug=False,
                   enable_asserts=False, num_devices=8)

    at_d = nc.dram_tensor("at8", [NMT, 128, 2, N], fp8, kind="ExternalInput")
    xs_d = nc.dram_tensor("xs", [T, 128, N], bf16, kind="ExternalInput")
    wa_d = nc.dram_tensor("wa", [128, 4 * C], bf16, kind="ExternalInput")
    bias_d = nc.dram_tensor("biasb", [128, C], f32, kind="ExternalInput")
    y_d = nc.dram_tensor("y", [T, N, C], f32, kind="ExternalOutput")

    with tile.TileContext(nc) as tc:
        with (
            tc.tile_pool(name="const", bufs=1) as constp,
            tc.tile_pool(name="atp", bufs=1) as atp,
            tc.tile_pool(name="bigslot", bufs=3) as slotp,
            tc.tile_pool(name="p12p", bufs=2) as p12p,
            tc.tile_pool(name="mp", bufs=2) as mp,
            tc.tile_pool(name="tmps", bufs=3) as tmps,
            tc.tile_pool(name="ystage", bufs=3) as ystage,
            tc.tile_pool(name="pps", bufs=4, space="PSUM") as pps,
            tc.tile_pool(name="sps", bufs=4, space="PSUM") as sps,
        ):
            def emit_body():
                _emit(nc, constp, atp, slotp, p12p, mp, tmps, ystage, pps, sps,
                      at_d, xs_d, wa_d, bias_d, y_d, with_bias)

            if repeat is None:
                emit_body()
            else:
                with tc.For_i(0, repeat, 1):
                    emit_body()

    nc.compile()
    _NC_CACHE[key] = nc
    return nc


def _emit(nc, constp, atp, slotp, p12p, mp, tmps, ystage, pps, sps,
          at_d, xs_d, wa_d, bias_d, y_d, with_bias):
    f32 = mybir.dt.float32
    bf16 = mybir.dt.bfloat16
    fp8 = mybir.dt.float8e4
    G = GROUPS[0]
    GW = G * C

    wa_t = constp.tile([128, 4 * C], bf16, tag="wa")
    bias_t = constp.tile([128, C], f32, tag="bias")
    nc.sync.dma_start(wa_t[:], wa_d[:, :])
    nc.sync.dma_start(bias_t[:], bias_d[:, :])

    # xs group tiles and p0 tiles share one 24KB/partition slot tag: at any
    # time at most 3 of {xs_g0, xs_g1, p0_g0, p0_g1} are live.
    xs_g = [slotp.tile([128, G, N], bf16, tag="big", name=f"xsg{g}")
            for g in range(2)]
    at_t = [atp.tile([128, 2, N], fp8, tag=f"at{mt}", name=f"at{mt}")
            for mt in range(NMT)]
    nc.sync.dma_start(xs_g[0][:], xs_d[0:G, :, :].rearrange("s p n -> p s n"))
    for mt in range(NMT):
        nc.sync.dma_start(at_t[mt][:], at_d[mt, :, :, :])
    nc.sync.dma_start(xs_g[1][:], xs_d[G:T, :, :].rearrange("s p n -> p s n"))

    def proj_site(g, p12, p0, idx, kp):
        """One projection site: slice idx of group g, node blocks 2kp,2kp+1."""
        cs = slice(idx * C, (idx + 1) * C)
        pp = pps.tile([128, 512], f32, tag="pp", name="pp")
        for j in range(2):
            k = 2 * kp + j
            nc.tensor.matmul(pp[:, j * 256:(j + 1) * 256],
                             xs_g[g][:, idx, k * 128:(k + 1) * 128],
                             wa_t[:], start=True, stop=True)
        # cols = k2*256 + pl*64 + c: pl 0=P1, 1=P2', 2=P0hi, 3=P0lo
        pv = pp.rearrange("p (k2 pl c) -> p pl k2 c", k2=2, pl=4, c=C)
        nc.vector.tensor_copy(p12[:, 0:2, 2 * kp:2 * kp + 2, cs],
                              pv[:, 0:2, :, :])
        # two PSUM operands in one op are rejected by walrus: copy hi (on the
        # otherwise-idle Scalar engine), then accumulate lo on DVE
        p0sl = p0[:, 2 * kp:2 * kp + 2, cs]
        nc.scalar.copy(p0sl, pv[:, 2, :, :])
        nc.vector.tensor_tensor(p0sl, p0sl, pv[:, 3, :, :],
                                op=mybir.AluOpType.add)

    def dr_chain(sp, k, moving):
        """One fp8 DoubleRow accumulation chain: sp += (2048*A)[kblk] @ moving.
        moving: [128, NB, GW] fp8 big tile."""
        for mt in range(NMT):
            nc.tensor.matmul(sp[:], at_t[mt][:, :, k * 128:(k + 1) * 128],
                             moving[:, 2 * mt:2 * mt + 2, :],
                             start=(mt == 0), stop=(mt == NMT - 1),
                             perf_mode=mybir.MatmulPerfMode.DoubleRow)

    def spmm2(p12, m, interleave=None):
        """Q' = A@P2' ; M = P1 - Q'."""
        nchunk = len(interleave) if interleave else 0
        for k in range(NB):
            if interleave and k % 4 == 0:
                c0 = (k // 4) * (nchunk // 4)
                c1 = (k // 4 + 1) * (nchunk // 4) if k < 12 else nchunk
                for thunk in interleave[c0:c1]:
                    thunk()
            sp = sps.tile([128, GW], f32, tag="sp", name="sp")
            dr_chain(sp, k, p12[:, 1, :, :])
            t = tmps.tile([128, GW], f32, tag="t", name="t")
            nc.scalar.mul(t[:], sp[:], 1.0 / ASCALE)
            nc.vector.tensor_tensor(m[:, k, :], p12[:, 0, k, :], t[:],
                                    op=mybir.AluOpType.subtract)

    def spmm3(g, m, p0, s0, interleave=None):
        """S = A@M ; y = P0 - S (+bias). `interleave`: list of thunks to
        emit spread through the k-loop (hides their DVE under spmm PE)."""
        nchunk = len(interleave) if interleave else 0
        for k in range(NB):
            if interleave and k % 4 == 0:
                c0 = (k // 4) * (nchunk // 4)
                c1 = (k // 4 + 1) * (nchunk // 4) if k < 12 else nchunk
                for thunk in interleave[c0:c1]:
                    thunk()
            sp = sps.tile([128, GW], f32, tag="sp", name="sp")
            dr_chain(sp, k, m)
            t = tmps.tile([128, GW], f32, tag="t", name="t")
            nc.scalar.mul(t[:], sp[:], 1.0 / ASCALE)
            yt = ystage.tile([128, GW], f32, tag="y", name="yt")
            nc.vector.tensor_sub(yt[:], p0[:, k, :], t[:])
            if with_bias:
                for idx in range(G):
                    ysl = yt[:, idx * C:(idx + 1) * C]
                    nc.vector.tensor_tensor(ysl, ysl, bias_t[:],
                                            op=mybir.AluOpType.add)
            dst = y_d[s0:s0 + G, k * 128:(k + 1) * 128, :]
            dst = dst.rearrange("s n c -> n s c")
            nc.sync.dma_start(dst, yt[:])

    # group 0 tiles; p12 plane 0 = P1, plane 1 = P2', both fp8
    p12_0 = p12p.tile([128, 2, NB, GW], fp8, tag="p12", name="p12_0")
    p0_0 = slotp.tile([128, NB, GW], f32, tag="big", name="p0_0")
    m_0 = mp.tile([128, NB, GW], fp8, tag="m", name="m_0")

    for idx in range(G):
        for kp in range(NB // 2):
            proj_site(0, p12_0, p0_0, idx, kp)

    # group 1 proj rides inside group 0's spmm2 phase (its PSUM evacuation
    # hides under the spmm chains); needs p12 bufs=2
    p12_1 = p12p.tile([128, 2, NB, GW], fp8, tag="p12", name="p12_1")
    p0_1 = slotp.tile([128, NB, GW], f32, tag="big", name="p0_1")
    m_1 = mp.tile([128, NB, GW], fp8, tag="m", name="m_1")
    proj1 = [(lambda i=idx, q=kp: proj_site(1, p12_1, p0_1, i, q))
             for idx in range(G) for kp in range(NB // 2)]
    spmm2(p12_0, m_0, interleave=proj1)

    # paired phase: spmm3(g0) and spmm2(g1) chains interleaved MM-by-MM so
    # consecutive matmuls share each A-tile's weight load
    DR = mybir.MatmulPerfMode.DoubleRow
    for k in range(NB):
        sp3 = sps.tile([128, GW], f32, tag="sp", name="sp3")
        sp2 = sps.tile([128, GW], f32, tag="sp", name="sp2")
        for mt in range(NMT):
            w = at_t[mt][:, :, k * 128:(k + 1) * 128]
            nc.tensor.matmul(sp3[:], w, m_0[:, 2 * mt:2 * mt + 2, :],
                             start=(mt == 0), stop=(mt == NMT - 1),
                             perf_mode=DR, skip_group_check=True)
            nc.tensor.matmul(sp2[:], w, p12_1[:, 1, 2 * mt:2 * mt + 2, :],
                             start=(mt == 0), stop=(mt == NMT - 1),
                             perf_mode=DR, skip_group_check=True)
        # y epilogue for group 0
        t = tmps.tile([128, GW], f32, tag="t", name="t")
        nc.scalar.mul(t[:], sp3[:], 1.0 / ASCALE)
        yt = ystage.tile([128, GW], f32, tag="y", name="yt")
        nc.vector.tensor_sub(yt[:], p0_0[:, k, :], t[:])
        if with_bias:
            for idx in range(G):
                ysl = yt[:, idx * C:(idx + 1) * C]
                nc.vector.tensor_tensor(ysl, ysl, bias_t[:],
                                        op=mybir.AluOpType.add)
        dst = y_d[0:G, k * 128:(k + 1) * 128, :]
        dst = dst.rearrange("s n c -> n s c")
        nc.sync.dma_start(dst, yt[:])
        # m epilogue for group 1
        t2 = tmps.tile([128, GW], f32, tag="t", name="t2")
        nc.scalar.mul(t2[:], sp2[:], 1.0 / ASCALE)
        nc.vector.tensor_tensor(m_1[:, k, :], p12_1[:, 0, k, :], t2[:],
                                op=mybir.AluOpType.subtract)

    spmm3(1, m_1, p0_1, G)


def _prep_inputs(x, A_norm, weight, bias):
    """Host-side shard + layout prep. Returns per-core input maps."""
    x = np.asarray(x, dtype=np.float32)
    A_norm = np.asarray(A_norm, dtype=np.float32)
    weight = np.asarray(weight, dtype=np.float32)
    bias = np.asarray(bias, dtype=np.float32)

    # DoubleRow A^T pack: at8[mt, k, i, n] = ASCALE * A[n, mt*256 + i*128 + k]
    AT = np.ascontiguousarray(A_norm.T)              # [m, n]
    at8 = AT.reshape(NMT, 2, 128, N).transpose(0, 2, 1, 3) * ASCALE
    at8_host = np.ascontiguousarray(at8).astype(FP8)

    W0, W1, W2 = weight[0], weight[1], weight[2]
    W02 = W0 - W2
    W02hi = W02.astype(BF16)
    W02lo = (W02 - W02hi.astype(np.float32)).astype(BF16)
    W1b = W1.astype(BF16)
    W2b = (2.0 * W2).astype(BF16)  # fold the Chebyshev 2x into W2
    # wa columns: [W1 | 2*W2 | W02hi | W02lo]; rows 0:64 hit Xhi, 64:128 Xlo
    wa_host = np.zeros((128, 4 * C), dtype=BF16)
    wa_host[0:C, 0:C] = W1b
    wa_host[C:2 * C, 0:C] = W1b
    wa_host[0:C, C:2 * C] = W2b
    wa_host[C:2 * C, C:2 * C] = W2b
    wa_host[0:C, 2 * C:3 * C] = W02hi
    wa_host[C:2 * C, 2 * C:3 * C] = W02hi
    wa_host[0:C, 3 * C:4 * C] = W02lo

    bias_host = np.ascontiguousarray(np.broadcast_to(bias, (128, C)),
                                     dtype=np.float32)

    in_maps = []
    for b in range(B):
        xt = np.ascontiguousarray(x[b].transpose(0, 2, 1))  # [T, C, N]
        hi = xt.astype(BF16)
        lo = (xt - hi.astype(np.float32)).astype(BF16)
        xs_host = np.concatenate([hi, lo], axis=1)          # [T, 128, N]
        in_maps.append({
            "at8": at8_host,
            "xs": np.ascontiguousarray(xs_host),
            "wa": wa_host,
            "biasb": bias_host,
        })
    return in_maps


def kernel(x, A_norm, weight, bias):
    with_bias = bool(np.any(np.asarray(bias)))
    nc = _build_nc(with_bias=with_bias)
    in_maps = _prep_inputs(x, A_norm, weight, bias)
    last_err = None
    for attempt in range(3):
        try:
            res = bass_utils.run_bass_kernel_spmd(nc, in_maps,
                                                  core_ids=list(range(8)))
            break
        except Exception as e:  # transient NRT_EXEC_UNIT_UNRECOVERABLE etc.
            last_err = e
            import time
            time.sleep(2.0 * (attempt + 1))
    else:
        raise last_err
    out = np.stack([res.results[b]["y"] for b in range(B)], axis=0)
    return out.astype(np.float32)
